# revision 33
# baseline (speedup 1.0000x reference)
"""Multi-head self-attention (B=2, S=2048, D=1024, H=16) on 8 TRN2 NeuronCores.

Sharding: core c handles batch b = c//4 and head group g = c%4 (4 heads each).
Each core computes qkv projection for its heads, masked-softmax attention, and
a partial output projection; the host sums the 4 partial outputs per batch.

Scores are computed transposed (keys on partitions, queries on the free dim) so
the P^T tile the PV matmul needs comes straight out of exp() with no transpose.
Softmax skips max-subtraction (scores are O(1) here); the denominator falls out
of a ones-column appended to the V stationary.

Mask handling: each 128key x 512query score tile is classified host-side as
skip / full / partial.  For partial tiles the leading fully-masked query
columns are sliced off the matmul moving dim entirely, and only the remaining
mixed region is multiplied by a 0/1 mask after exp.

Schedule notes (v3):
- PE warm-up: dummy matmuls run during the initial DMA wait so the tensor
  engine's DVFS p-state is at full clock when the first real matmul issues;
  small dummy trickles between phase-1 kt sections keep the clock up while
  the input stream catches up (any idle gap drops the PE to half clock for
  the next ~3us).
- Attention works in supergroups of 4 key tiles: scores for the whole group
  land in one 4-bank PSUM tile (bufs=1), one exp() covers the group
  (tight-packed, so partial tiles cost no extra activate), then 4 PV
  matmuls.  This halves the Act instruction count; Act drops to ~50% duty
  and stops pacing the PE.
- Valid (unmasked) tiles run before partial tiles within each head-strip so
  the first PV of a head never waits on a mask multiply that is queued
  behind the previous head's normalize chain on the DVE queue.
- Tail: strip-3 output projection casts alternate Act/DVE so the two PSUM
  buffers drain fast enough to keep the matmul pairs back-to-back, and each
  512-token block DMAs out as soon as both column halves are done.
"""

from contextlib import ExitStack

import numpy as np

import concourse.bass as bass
import concourse.tile as tile
from concourse import bacc, mybir
from concourse.bass_utils import run_bass_kernel_spmd

F32 = mybir.dt.float32
F16 = mybir.dt.float16

B, S, D, H, DH = 2, 2048, 1024, 16, 64
HPC = 4          # heads per core
NCORES = 8
KT = S // 128    # 16 key tiles of 128
QS = S // 512    # 4 query strips of 512
DKT = D // 128   # 8 contraction tiles for the projections


def _to_f16(x):
    return np.ascontiguousarray(x).astype(np.float16)


def _build(spec, uregw, debug=False):
    """spec[qs][ki] is ('s',) skip | ('f',) full | ('p', pref, regw, mid)."""
    nc = bacc.Bacc()

    mw = sum(uregw) if uregw else 1
    wqc = [nc.dram_tensor(f"wqc{i}", [128, 2, 512], F16, kind="ExternalInput")
           for i in range(4)]
    xtc = [nc.dram_tensor(f"xtc{i}", [128, 2, 512], F16, kind="ExternalInput")
           for i in range(4)]
    xrd = [nc.dram_tensor(f"xr{i}", [128, DKT, 512], F16, kind="ExternalInput")
           for i in range(1, 4)]
    wv = nc.dram_tensor("wv", [128, DKT, 256], F16, kind="ExternalInput")
    wo = nc.dram_tensor("wo", [128, 2, D], F16, kind="ExternalInput")
    maskp = nc.dram_tensor("maskp", [128, mw], F16, kind="ExternalInput")
    out = nc.dram_tensor("out", [128, 4 * QS, D], F16, kind="ExternalOutput")

    with tile.TileContext(nc) as tc, ExitStack() as top:
        persist = top.enter_context(tc.tile_pool(name="persist", bufs=1))

        # ---- persistent tiles ----
        qk = [persist.tile([128, S], F16, name=f"qk{ct}", tag=f"qk{ct}") for ct in range(4)]
        v_ext = [
            persist.tile([128, 4 * HPC * 65], F16, name=f"vx{s}", tag=f"vx{s}")
            for s in range(QS)
        ]
        ot = [
            [
                persist.tile([128, 512], F16, name=f"ot{t}_{s}", tag=f"ot{t}_{s}")
                for s in range(QS)
            ]
            for t in range(2)
        ]
        wo_t = persist.tile([128, 2, D], F16, tag="wo")
        mtiles = [
            persist.tile([128, uregw[m]], F16, name=f"mt{m}", tag=f"mt{m}")
            for m in range(len(uregw))
        ]
        osb = [
            persist.tile([128, 4, D], F16, name=f"osb{i}", tag=f"osb{i}")
            for i in range(2)
        ]
        # dummy warm-up source (memset once, read-only afterwards)
        dum = persist.tile([128, 256], F16, tag="dum")

        # phase-1 input tiles
        xw = top.enter_context(tc.tile_pool(name="xw", bufs=1))
        xt0 = xw.tile([128, DKT, 512], F16, tag="xt0")
        xr = [
            xw.tile([128, DKT, 512], F16, name=f"xr{i}", tag=f"xr{i}")
            for i in range(1, 4)
        ]
        wqk_t = xw.tile([128, DKT, 512], F16, tag="wqk")
        wv_t = xw.tile([128, DKT, 256], F16, tag="wv")

        # ---- input DMAs ----
        nc.gpsimd.memset(dum[:], 0.0)
        # ones columns of v_ext, generated on-device (a DMA here costs 8192
        # 2-byte descriptors that jam the queues for ~15us)
        for s in range(QS):
            nc.gpsimd.memset(
                v_ext[s][:].rearrange("p (g c) -> p g c", c=65)[:, :, 64:65], 1.0
            )
        # phase-1 stream: the first two kt pairs go on the hardware queues
        # (they start ~5us sooner but only sustain ~80 GB/s), the rest rides
        # the software-dynamic (gpsimd) queue, which needs ~10us to spin up
        # but then sustains ~220 GB/s - the handoff lands just as the kt
        # loop finishes the hardware-queue pairs
        nc.sync.dma_start(wqk_t[:, 0:2, :], wqc[0][:])
        nc.scalar.dma_start(xt0[:, 0:2, :], xtc[0][:])
        nc.sync.dma_start(wqk_t[:, 2:4, :], wqc[1][:])
        nc.scalar.dma_start(xt0[:, 2:4, :], xtc[1][:])
        for i in range(2, 4):
            nc.gpsimd.dma_start(wqk_t[:, 2 * i : 2 * i + 2, :], wqc[i][:])
            nc.gpsimd.dma_start(xt0[:, 2 * i : 2 * i + 2, :], xtc[i][:])
        nc.gpsimd.dma_start(xr[0][:], xrd[0][:])
        nc.scalar.dma_start(wv_t[:], wv[:])
        nc.scalar.dma_start(wo_t[:], wo[:])
        nc.sync.dma_start(xr[1][:], xrd[1][:])
        nc.scalar.dma_start(xr[2][:], xrd[2][:])
        moff = 0
        for m in range(len(uregw)):
            nc.sync.dma_start(mtiles[m][:], maskp[:, moff : moff + uregw[m]])
            moff += uregw[m]

        def xslice(kt, lo, hi):
            if hi <= 512:
                return xt0[:, kt, lo:hi]
            c = lo // 512
            return xr[c - 1][:, kt, lo - 512 * c : hi - 512 * c]

        # ---- phase 1: q/k for strip 0, kt-outer; warm-up dummies ramp the
        # PE clock before the first chunk lands, trickles keep it up when the
        # DMA stream briefly falls behind the kt loop ----
        with ExitStack() as pha:
            psA = pha.enter_context(tc.tile_pool(name="psA", bufs=1, space="PSUM"))
            wp = psA.tile([128, 512], F32, name="warm", tag="warm")

            def dummy(n, cols=256):
                for _ in range(n):
                    nc.tensor.matmul(
                        wp[:, 0:cols], dum[:, 0:128], dum[:, 0:cols],
                        start=True, stop=True,
                    )

            dummy(16)
            dummy(12, cols=64)
            pa = [
                psA.tile([128, 512], F32, name=f"pa{ct}", tag=f"pa{ct}")
                for ct in range(4)
            ]
            for kt in range(DKT):
                for ct in range(4):
                    nc.tensor.matmul(
                        pa[ct][:],
                        wqk_t[:, kt, 128 * ct : 128 * ct + 128],
                        xt0[:, kt, :],
                        start=(kt == 0),
                        stop=(kt == DKT - 1),
                    )
                if kt % 2 == 1 and kt < DKT - 1:
                    # cover the gap until the next kt-pair chunk lands
                    dummy(8, cols=64)
            for ct in range(4):
                nc.vector.tensor_copy(qk[ct][:, 0:512], pa[ct][:])

        # ---- phase 2: attention in supergroups of 4 key tiles, remaining
        # projection work interleaved as PE filler ----
        with ExitStack() as ph2:
            ptp = ph2.enter_context(tc.tile_pool(name="pt", bufs=4))
            nrm = ph2.enter_context(tc.tile_pool(name="nrm", bufs=3))
            ps_st = ph2.enter_context(
                tc.tile_pool(name="ps_st", bufs=2, space="PSUM")
            )
            ps_o = ph2.enter_context(tc.tile_pool(name="ps_o", bufs=2, space="PSUM"))
            fillp = ph2.enter_context(tc.tile_pool(name="fillp", bufs=2, space="PSUM"))

            def emit_v(st, eng=None):
                ps = fillp.tile([128, 512], F32, tag="fill")
                for kt in range(DKT):
                    nc.tensor.matmul(
                        ps[:, 0:256],
                        xslice(kt, 128 * st, 128 * st + 128),
                        wv_t[:, kt, :],
                        start=(kt == 0),
                        stop=(kt == DKT - 1),
                    )
                j = st % 4
                dst = v_ext[st // 4][:, 260 * j : 260 * j + 260].rearrange(
                    "p (h c) -> p h c", c=65
                )[:, :, 0:64]
                nc.vector.tensor_copy(
                    dst, ps[:, 0:256].rearrange("p (h c) -> p h c", c=64)
                )

            def emit_qk(ss, ct, eng=None):
                # ct 0-1 produce q for strip ss (needed when strip ss opens);
                # ct 2-3 produce k for the keys of strip ss (first read only
                # when a query strip's supergroups reach those key tiles) -
                # so the k units of the last strip can run as filler inside
                # that strip itself
                ps = fillp.tile([128, 512], F32, tag="fill")
                lo = 512 * ss
                for kt in range(DKT):
                    nc.tensor.matmul(
                        ps[:],
                        wqk_t[:, kt, 128 * ct : 128 * ct + 128],
                        xslice(kt, lo, lo + 512),
                        start=(kt == 0),
                        stop=(kt == DKT - 1),
                    )
                nc.vector.tensor_copy(qk[ct][:, lo : lo + 512], ps[:])

            def emit_oproj(qs, sti, oc, eng=None):
                ob = osb[qs % 2]
                pop = fillp.tile([128, 512], F32, tag="fill")
                for t in range(2):
                    nc.tensor.matmul(
                        pop[:],
                        ot[t][qs][:, 128 * sti : 128 * sti + 128],
                        wo_t[:, t, 512 * oc : 512 * oc + 512],
                        start=(t == 0),
                        stop=(t == 1),
                    )
                dst = ob[:, sti, 512 * oc : 512 * oc + 512]
                if eng == "scalar":
                    nc.scalar.copy(dst, pop[:])
                else:
                    nc.vector.tensor_copy(dst, pop[:])
                if sti == 3 and oc == 1:
                    nc.sync.dma_start(out[:, 4 * qs : 4 * qs + 4, :], ob[:])

            def mk(f, *a):
                return lambda **kw: f(*a, **kw)

            # v for keys 0-511 must precede attention strip 0
            for st in range(4):
                emit_v(st)

            tail_pops = []

            fills = {
                0: [mk(emit_qk, 1, ct) for ct in range(4)]
                + [mk(emit_v, st) for st in range(4, 8)],
                1: [mk(emit_qk, 2, ct) for ct in range(4)]
                + [mk(emit_v, st) for st in range(8, 12)]
                + [mk(emit_oproj, 0, sti, oc) for sti in range(4) for oc in range(2)],
                2: [mk(emit_qk, 3, ct) for ct in range(2)]
                + [mk(emit_oproj, 1, sti, oc) for sti in range(4) for oc in range(2)],
                # strip 3 would otherwise run nearly filler-dry for 36 slots,
                # exposing the exp latency at every group boundary (and each
                # tiny gap drops the PE p-state): its own k tiles, the last v
                # strip and strip-2's output projection keep it fed
                3: [mk(emit_qk, 3, ct) for ct in range(2, 4)]
                + [mk(emit_v, st) for st in range(12, 16)]
                + [mk(emit_oproj, 2, sti, oc) for sti in range(4) for oc in range(2)],
            }

            for qs in range(QS):
                partials = []   # (ki, pref, regw, mid)
                valids = []
                for ki in range(KT):
                    st = spec[qs][ki]
                    if st[0] == "f":
                        valids.append(ki)
                    elif st[0] == "p":
                        partials.append((ki, st[1], st[2], st[3]))
                partials.sort(key=lambda t: t[1])
                # valid tiles first: the opening PV write must span the full
                # strip (pref 0), and partial tiles' mask multiplies then sit
                # well clear of the PV matmuls that consume them
                tiles = [(ki, 0, 0, -1) for ki in valids] + partials
                assert tiles and tiles[0][1] == 0, "need a full-width opener"
                sgs = [tiles[i : i + 2] for i in range(0, len(tiles), 2)]
                n_tiles = len(tiles)
                fq = fills[qs]
                n_slots = HPC * (len(sgs) + 1)
                reserve = 4 if qs == QS - 1 else 0
                # strip 0's fillers read x tokens 512-1023, which are still
                # in flight on the DMA stream when the strip opens
                defer = 4 if qs == 0 else 0
                # spread the filler pops evenly across the strip's slots: a
                # slot left dry exposes the exp latency AND drops the PE
                # p-state to half clock for the next ~3us.  Strip 3's first 6
                # units write the k tiles / v strip its own supergroups 6-7
                # read - they MUST be emitted before head 0 reaches slot 6,
                # so they front-load into slots 0-5.
                front = 6 if qs == QS - 1 else 0
                navail = max(len(fq) - reserve - front, 0)
                span = n_slots - defer - front
                sched = [0] * n_slots
                for i in range(front):
                    sched[i] += 1
                for i in range(navail):
                    sched[defer + front + i * span // max(navail, 1)] += 1
                slot = 0

                for h in range(HPC):
                    qT = qk[h // 2][64 * (h % 2) : 64 * (h % 2) + 64, :]
                    kT = qk[2 + h // 2][64 * (h % 2) : 64 * (h % 2) + 64, :]
                    po = ps_o.tile([65, 512], F32, tag="po")

                    pending = None
                    npv = 0
                    for g in range(len(sgs) + 1):
                        jobs = None
                        if g < len(sgs):
                            sg = sgs[g]
                            pst = ps_st.tile([128, 1024], F32, tag="pst")
                            pt = ptp.tile([128, 1024], F16, tag="pt")
                            jobs = []
                            off = 0
                            for ki, pref, regw, mid in sg:
                                w = 512 - pref
                                if off // 512 != (off + w - 1) // 512:
                                    # a single matmul's PSUM write must stay
                                    # inside one 2KB bank
                                    off = (off + 511) // 512 * 512
                                nc.tensor.matmul(
                                    pst[:, off : off + w],
                                    kT[:, 128 * ki : 128 * ki + 128],
                                    qT[:, 512 * qs + pref : 512 * qs + 512],
                                    start=True,
                                    stop=True,
                                )
                                jobs.append((pt, off, w, ki, pref, regw, mid))
                                off += w
                            # tight packing keeps this a single exp even for
                            # the partial (diagonal) pairs
                            nc.scalar.activation(
                                pt[:, 0:off],
                                pst[:, 0:off],
                                mybir.ActivationFunctionType.Exp,
                            )
                            # mask multiplies ride the Pool engine except in
                            # strip 0: valids-first puts them at the END of
                            # each head there's >=1 valid group, so the Pool
                            # queue's head-of-line normalize broadcast has
                            # long resolved; strip 0 (all-partial) needs them
                            # at head start, where that broadcast would stall
                            # them on DVE they only sit behind fast casts
                            meng = nc.vector if qs == 0 else nc.gpsimd
                            for pt_, o, w, ki, pref, regw, mid in jobs:
                                if mid >= 0:
                                    meng.tensor_mul(
                                        pt_[:, o : o + regw],
                                        pt_[:, o : o + regw],
                                        mtiles[mid][:],
                                    )
                        for _ in range(min(sched[slot], max(len(fq) - reserve, 0))):
                            fq.pop(0)()
                        slot += 1
                        if pending is not None:
                            for pt_, o, w, ki, pref, regw, mid in pending:
                                vcol = 260 * (ki % 4) + 65 * h
                                nc.tensor.matmul(
                                    po[:, pref : pref + w],
                                    v_ext[ki // 4][:, vcol : vcol + 65],
                                    pt_[:, o : o + w],
                                    start=(npv == 0),
                                    stop=(npv == n_tiles - 1),
                                )
                                npv += 1
                        pending = jobs

                    if h == HPC - 1:
                        # flush leftover filler BEFORE the normalize chain
                        flush_engs = ["scalar", "vector"]
                        fi = 0
                        while fq:
                            fq.pop(0)(eng=flush_engs[fi % 2])
                            fi += 1
                        if qs == QS - 1:
                            # open the first two tail accumulations: their
                            # t=0 operand (heads 0-1) is long ready, and the
                            # 4 matmuls keep the PE clocked through the
                            # ~3.5us final normalize chain
                            for sti in range(2):
                                pop = ps_st.tile([128, 1024], F32, tag="pst")
                                for oc in range(2):
                                    nc.tensor.matmul(
                                        pop[:, 512 * oc : 512 * oc + 512],
                                        ot[0][3][:, 128 * sti : 128 * sti + 128],
                                        wo_t[:, 0, 512 * oc : 512 * oc + 512],
                                        start=True,
                                        stop=False,
                                    )
                                tail_pops.append(pop)
                    # normalize: row 64 of po is the softmax denominator
                    # (copy to SBUF first: the custom-DVE reciprocal misreads
                    # PSUM operands on hardware)
                    rden = nrm.tile([1, 512], F32, tag="rden")
                    nc.vector.tensor_copy(rden[:], po[64:65, :])
                    rrec = nrm.tile([1, 512], F32, tag="rrec")
                    nc.vector.reciprocal_approx_fast(rrec[:], rden[:])
                    rb = nrm.tile([64, 512], F32, tag="rb")
                    nc.gpsimd.partition_broadcast(rb[:], rrec[:])
                    nc.vector.tensor_mul(
                        ot[h // 2][qs][64 * (h % 2) : 64 * (h % 2) + 64, :],
                        po[0:64, :],
                        rb[:],
                    )

                while fq:
                    fq.pop(0)()

            # strip-3 output projection: each 512-token block's two column
            # halves accumulate in one rotating [128,1024] score buffer
            # (blocks 0-1 were opened before the final normalize), casts
            # alternate Act/DVE, and every block DMAs out on completion
            ob = osb[1]
            for sti in range(4):
                if sti < 2:
                    pop = tail_pops[sti]
                    for oc in range(2):
                        nc.tensor.matmul(
                            pop[:, 512 * oc : 512 * oc + 512],
                            ot[1][3][:, 128 * sti : 128 * sti + 128],
                            wo_t[:, 1, 512 * oc : 512 * oc + 512],
                            start=False,
                            stop=True,
                        )
                else:
                    pop = ps_st.tile([128, 1024], F32, tag="pst")
                    for oc in range(2):
                        for t in range(2):
                            nc.tensor.matmul(
                                pop[:, 512 * oc : 512 * oc + 512],
                                ot[t][3][:, 128 * sti : 128 * sti + 128],
                                wo_t[:, t, 512 * oc : 512 * oc + 512],
                                start=(t == 0),
                                stop=(t == 1),
                            )
                nc.scalar.copy(ob[:, sti, 0:512], pop[:, 0:512])
                nc.vector.tensor_copy(ob[:, sti, 512:1024], pop[:, 512:1024])
                nc.sync.dma_start(
                    out[:, 12 + sti : 13 + sti, :], ob[:, sti : sti + 1, :]
                )

    nc.finalize()
    return nc


_cache = {}


def _get_nc(key):
    if key not in _cache:
        spec, uregw = key
        _cache[key] = _build([list(r) for r in spec], list(uregw))
    return _cache[key]


def _tile_km(a):
    """[K*128, w] -> [128, K, w] partition-major contiguous."""
    k1, w = a.shape
    return np.ascontiguousarray(
        a.reshape(k1 // 128, 128, w).transpose(1, 0, 2)
    )


def _prepare(x, mask, w_qkv, w_out):
    """Host-side sharding. Returns (cache_key, in_maps)."""
    scale = 1.0 / np.sqrt(DH)

    keeps = [(mask[b] != 0).T.astype(np.float32) for b in range(B)]  # [k, q]
    keep_any = np.maximum.reduce(keeps)
    keep_all = np.minimum.reduce(keeps)

    uniq = {}
    uregw = []
    umask = []
    spec = []
    for qs in range(QS):
        row = []
        for ki in range(KT):
            blk_any = keep_any[128 * ki : 128 * ki + 128, 512 * qs : 512 * qs + 512]
            blk_all = keep_all[128 * ki : 128 * ki + 128, 512 * qs : 512 * qs + 512]
            if blk_any.max() == 0.0:
                row.append(("s",))
                continue
            if blk_all.min() == 1.0:
                row.append(("f",))
                continue
            colm = blk_any.max(axis=0)
            colv = blk_all.min(axis=0)
            nz = np.nonzero(colm)[0]
            pref = int(nz[0]) if len(nz) else 512
            mixed = np.nonzero(colv == 0)[0]
            end = int(mixed[-1]) + 1 if len(mixed) else pref
            regw = max(end - pref, 1)
            regs = tuple(
                k[128 * ki : 128 * ki + 128, 512 * qs + pref : 512 * qs + pref + regw]
                .astype(np.float16)
                .tobytes()
                for k in keeps
            )
            if regs not in uniq:
                uniq[regs] = len(uregw)
                uregw.append(regw)
                umask.append(
                    [
                        np.frombuffer(r, np.float16).reshape(128, regw)
                        for r in regs
                    ]
                )
            row.append(("p", pref, regw, uniq[regs]))
        spec.append(tuple(row))
    key = (tuple(spec), tuple(uregw))

    in_maps = []
    for c in range(NCORES):
        b, g = c // 4, c % 4
        heads = range(4 * g, 4 * g + 4)
        xT = _tile_km(_to_f16(x[b].T))            # [128, 8, 2048]
        wq = np.concatenate(
            [w_qkv[:, 64 * h : 64 * h + 64] for h in heads], axis=1
        ) * scale
        wk = np.concatenate(
            [w_qkv[:, D + 64 * h : D + 64 * h + 64] for h in heads], axis=1
        )
        wvv = np.concatenate(
            [w_qkv[:, 2 * D + 64 * h : 2 * D + 64 * h + 64] for h in heads], axis=1
        )
        woo = np.concatenate(
            [w_out[64 * h : 64 * h + 64, :] for h in heads], axis=0
        )
        wqk = _tile_km(_to_f16(np.concatenate([wq, wk], axis=1)))  # [128, 8, 512]
        if umask:
            mk = np.concatenate([r[b] for r in umask], axis=1).astype(np.float16)
        else:
            mk = np.zeros((128, 1), np.float16)
        im = {
            "wv": _tile_km(_to_f16(wvv)),
            "wo": _tile_km(_to_f16(np.ascontiguousarray(woo))),
            "maskp": np.ascontiguousarray(mk),
        }
        for i in range(4):
            im[f"wqc{i}"] = np.ascontiguousarray(wqk[:, 2 * i : 2 * i + 2, :])
            im[f"xtc{i}"] = np.ascontiguousarray(xT[:, 2 * i : 2 * i + 2, 0:512])
        for i in range(1, 4):
            im[f"xr{i}"] = np.ascontiguousarray(xT[:, :, 512 * i : 512 * i + 512])
        in_maps.append(im)
    return key, in_maps


def _unshuffle_out(o):
    """[128, 16, D] tile-major kernel output -> [S, D]."""
    return np.ascontiguousarray(o.transpose(1, 0, 2)).reshape(S, D)


def _run(x, mask, w_qkv, w_out, trace=False, trace_cores=None):
    key, in_maps = _prepare(x, mask, w_qkv, w_out)
    nc = _get_nc(key)
    res = run_bass_kernel_spmd(
        nc,
        in_maps,
        core_ids=list(range(NCORES)),
        trace=trace,
        trace_cores=trace_cores,
    )
    outs = np.stack(
        [
            sum(
                _unshuffle_out(res.results[4 * b + g]["out"].astype(np.float32))
                for g in range(4)
            )
            for b in range(B)
        ]
    )
    return outs.astype(np.float32), res


def kernel(x, mask, w_qkv, w_out):
    x = np.asarray(x, np.float32)
    mask = np.asarray(mask)
    w_qkv = np.asarray(w_qkv, np.float32)
    w_out = np.asarray(w_out, np.float32)
    out, _ = _run(x, mask, w_qkv, w_out)
    return out


# revision 35
# speedup vs baseline: 1.6380x; 1.6380x over previous
"""Multi-head self-attention (B=2, S=2048, D=1024, H=16) on 8 TRN2 NeuronCores.

Sharding: core c handles batch b = c//4 and head group g = c%4 (4 heads each).
Each core computes qkv projection for its heads, masked-softmax attention, and
a partial output projection; the host sums the 4 partial outputs per batch.

Scores are computed transposed (keys on partitions, queries on the free dim) so
the P^T tile the PV matmul needs comes straight out of exp() with no transpose.
Softmax skips max-subtraction (scores are O(1) here); the denominator falls out
of a ones-column appended to the V stationary.

Mask handling: each 128key x 512query score tile is classified host-side as
skip / full / partial.  For partial tiles the leading fully-masked query
columns are sliced off the matmul moving dim entirely, and only the remaining
mixed region is multiplied by a 0/1 mask after exp.

Schedule notes (v3):
- PE warm-up: dummy matmuls run during the initial DMA wait so the tensor
  engine's DVFS p-state is at full clock when the first real matmul issues;
  small dummy trickles between phase-1 kt sections keep the clock up while
  the input stream catches up (any idle gap drops the PE to half clock for
  the next ~3us).
- Attention works in supergroups of 4 key tiles: scores for the whole group
  land in one 4-bank PSUM tile (bufs=1), one exp() covers the group
  (tight-packed, so partial tiles cost no extra activate), then 4 PV
  matmuls.  This halves the Act instruction count; Act drops to ~50% duty
  and stops pacing the PE.
- Valid (unmasked) tiles run before partial tiles within each head-strip so
  the first PV of a head never waits on a mask multiply that is queued
  behind the previous head's normalize chain on the DVE queue.
- Tail: strip-3 output projection casts alternate Act/DVE so the two PSUM
  buffers drain fast enough to keep the matmul pairs back-to-back, and each
  512-token block DMAs out as soon as both column halves are done.
"""

from contextlib import ExitStack

import numpy as np

import concourse.bass as bass
import concourse.tile as tile
from concourse import bacc, mybir
from concourse.bass_utils import run_bass_kernel_spmd

F32 = mybir.dt.float32
F16 = mybir.dt.float16

B, S, D, H, DH = 2, 2048, 1024, 16, 64
HPC = 4          # heads per core
NCORES = 8
KT = S // 128    # 16 key tiles of 128
QS = S // 512    # 4 query strips of 512
DKT = D // 128   # 8 contraction tiles for the projections


def _to_f16(x):
    return np.ascontiguousarray(x).astype(np.float16)


def _build(spec, uregw, debug=False):
    """spec[qs][ki] is ('s',) skip | ('f',) full | ('p', pref, regw, mid)."""
    nc = bacc.Bacc()

    mw = sum(uregw) if uregw else 1
    wqc = [nc.dram_tensor(f"wqc{i}", [128, 2, 512], F16, kind="ExternalInput")
           for i in range(4)]
    xtc = [nc.dram_tensor(f"xtc{i}", [128, 2, 512], F16, kind="ExternalInput")
           for i in range(4)]
    xrd = [nc.dram_tensor(f"xr{i}", [128, DKT, 512], F16, kind="ExternalInput")
           for i in range(1, 4)]
    wv = nc.dram_tensor("wv", [128, DKT, 256], F16, kind="ExternalInput")
    wo = nc.dram_tensor("wo", [128, 2, D], F16, kind="ExternalInput")
    maskp = nc.dram_tensor("maskp", [128, mw], F16, kind="ExternalInput")
    out = nc.dram_tensor("out", [128, 4 * QS, D], F16, kind="ExternalOutput")

    with tile.TileContext(nc) as tc, ExitStack() as top:
        persist = top.enter_context(tc.tile_pool(name="persist", bufs=1))

        # ---- persistent tiles ----
        qk = [persist.tile([128, S], F16, name=f"qk{ct}", tag=f"qk{ct}") for ct in range(4)]
        v_ext = [
            persist.tile([128, 4 * HPC * 65], F16, name=f"vx{s}", tag=f"vx{s}")
            for s in range(QS)
        ]
        ot = [
            [
                persist.tile([128, 512], F16, name=f"ot{t}_{s}", tag=f"ot{t}_{s}")
                for s in range(QS)
            ]
            for t in range(2)
        ]
        wo_t = persist.tile([128, 2, D], F16, tag="wo")
        mtiles = [
            persist.tile([128, uregw[m]], F16, name=f"mt{m}", tag=f"mt{m}")
            for m in range(len(uregw))
        ]
        osb = [
            persist.tile([128, 4, D], F16, name=f"osb{i}", tag=f"osb{i}")
            for i in range(2)
        ]
        # dummy warm-up source (memset once, read-only afterwards)
        dum = persist.tile([128, 256], F16, tag="dum")

        # phase-1 input tiles
        xw = top.enter_context(tc.tile_pool(name="xw", bufs=1))
        xt0 = xw.tile([128, DKT, 512], F16, tag="xt0")
        xr = [
            xw.tile([128, DKT, 512], F16, name=f"xr{i}", tag=f"xr{i}")
            for i in range(1, 4)
        ]
        wqk_t = xw.tile([128, DKT, 512], F16, tag="wqk")
        wv_t = xw.tile([128, DKT, 256], F16, tag="wv")

        # ---- input DMAs ----
        nc.gpsimd.memset(dum[:], 0.0)
        # ones columns of v_ext, generated on-device (a DMA here costs 8192
        # 2-byte descriptors that jam the queues for ~15us)
        for s in range(QS):
            nc.gpsimd.memset(
                v_ext[s][:].rearrange("p (g c) -> p g c", c=65)[:, :, 64:65], 1.0
            )
        # phase-1 stream: the first two kt pairs go on the hardware queues
        # (they start ~5us sooner but only sustain ~80 GB/s), the rest rides
        # the software-dynamic (gpsimd) queue, which needs ~10us to spin up
        # but then sustains ~220 GB/s - the handoff lands just as the kt
        # loop finishes the hardware-queue pairs
        nc.sync.dma_start(wqk_t[:, 0:2, :], wqc[0][:])
        nc.scalar.dma_start(xt0[:, 0:2, :], xtc[0][:])
        nc.sync.dma_start(wqk_t[:, 2:4, :], wqc[1][:])
        nc.scalar.dma_start(xt0[:, 2:4, :], xtc[1][:])
        for i in range(2, 4):
            nc.gpsimd.dma_start(wqk_t[:, 2 * i : 2 * i + 2, :], wqc[i][:])
            nc.gpsimd.dma_start(xt0[:, 2 * i : 2 * i + 2, :], xtc[i][:])
        nc.gpsimd.dma_start(xr[0][:], xrd[0][:])
        nc.scalar.dma_start(wv_t[:], wv[:])
        nc.scalar.dma_start(wo_t[:], wo[:])
        nc.sync.dma_start(xr[1][:], xrd[1][:])
        nc.scalar.dma_start(xr[2][:], xrd[2][:])
        moff = 0
        for m in range(len(uregw)):
            nc.sync.dma_start(mtiles[m][:], maskp[:, moff : moff + uregw[m]])
            moff += uregw[m]

        def xslice(kt, lo, hi):
            if hi <= 512:
                return xt0[:, kt, lo:hi]
            c = lo // 512
            return xr[c - 1][:, kt, lo - 512 * c : hi - 512 * c]

        # ---- phase 1: q/k for strip 0, kt-outer; warm-up dummies ramp the
        # PE clock before the first chunk lands, trickles keep it up when the
        # DMA stream briefly falls behind the kt loop ----
        with ExitStack() as pha:
            psA = pha.enter_context(tc.tile_pool(name="psA", bufs=1, space="PSUM"))
            wp = psA.tile([128, 512], F32, name="warm", tag="warm")

            def dummy(n, cols=256):
                for _ in range(n):
                    nc.tensor.matmul(
                        wp[:, 0:cols], dum[:, 0:128], dum[:, 0:cols],
                        start=True, stop=True,
                    )

            dummy(16)
            dummy(12, cols=64)
            pa = [
                psA.tile([128, 512], F32, name=f"pa{ct}", tag=f"pa{ct}")
                for ct in range(4)
            ]
            for kt in range(DKT):
                for ct in range(4):
                    nc.tensor.matmul(
                        pa[ct][:],
                        wqk_t[:, kt, 128 * ct : 128 * ct + 128],
                        xt0[:, kt, :],
                        start=(kt == 0),
                        stop=(kt == DKT - 1),
                    )
                if kt % 2 == 1 and kt < DKT - 1:
                    # cover the gap until the next kt-pair chunk lands
                    dummy(8, cols=64)
            for ct in range(4):
                nc.vector.tensor_copy(qk[ct][:, 0:512], pa[ct][:])

        # ---- phase 2: attention in supergroups of 4 key tiles, remaining
        # projection work interleaved as PE filler ----
        with ExitStack() as ph2:
            ptp = ph2.enter_context(tc.tile_pool(name="pt", bufs=4))
            nrm = ph2.enter_context(tc.tile_pool(name="nrm", bufs=3))
            ps_st = ph2.enter_context(
                tc.tile_pool(name="ps_st", bufs=2, space="PSUM")
            )
            ps_o = ph2.enter_context(tc.tile_pool(name="ps_o", bufs=2, space="PSUM"))
            fillp = ph2.enter_context(tc.tile_pool(name="fillp", bufs=2, space="PSUM"))

            def emit_v(st, eng=None):
                ps = fillp.tile([128, 512], F32, tag="fill")
                for kt in range(DKT):
                    nc.tensor.matmul(
                        ps[:, 0:256],
                        xslice(kt, 128 * st, 128 * st + 128),
                        wv_t[:, kt, :],
                        start=(kt == 0),
                        stop=(kt == DKT - 1),
                    )
                j = st % 4
                dst = v_ext[st // 4][:, 260 * j : 260 * j + 260].rearrange(
                    "p (h c) -> p h c", c=65
                )[:, :, 0:64]
                nc.vector.tensor_copy(
                    dst, ps[:, 0:256].rearrange("p (h c) -> p h c", c=64)
                )

            def emit_qk(ss, ct, eng=None):
                # ct 0-1 produce q for strip ss (needed when strip ss opens);
                # ct 2-3 produce k for the keys of strip ss (first read only
                # when a query strip's supergroups reach those key tiles) -
                # so the k units of the last strip can run as filler inside
                # that strip itself
                ps = fillp.tile([128, 512], F32, tag="fill")
                lo = 512 * ss
                for kt in range(DKT):
                    nc.tensor.matmul(
                        ps[:],
                        wqk_t[:, kt, 128 * ct : 128 * ct + 128],
                        xslice(kt, lo, lo + 512),
                        start=(kt == 0),
                        stop=(kt == DKT - 1),
                    )
                nc.vector.tensor_copy(qk[ct][:, lo : lo + 512], ps[:])

            def emit_oproj(qs, sti, oc, eng=None):
                ob = osb[qs % 2]
                pop = fillp.tile([128, 512], F32, tag="fill")
                for t in range(2):
                    nc.tensor.matmul(
                        pop[:],
                        ot[t][qs][:, 128 * sti : 128 * sti + 128],
                        wo_t[:, t, 512 * oc : 512 * oc + 512],
                        start=(t == 0),
                        stop=(t == 1),
                    )
                dst = ob[:, sti, 512 * oc : 512 * oc + 512]
                if eng == "scalar":
                    nc.scalar.copy(dst, pop[:])
                else:
                    nc.vector.tensor_copy(dst, pop[:])
                if sti == 3 and oc == 1:
                    nc.sync.dma_start(out[:, 4 * qs : 4 * qs + 4, :], ob[:])

            def mk(f, *a):
                return lambda **kw: f(*a, **kw)

            # v for keys 0-511 must precede attention strip 0
            for st in range(4):
                emit_v(st)

            tail_pops = []

            fills = {
                0: [mk(emit_qk, 1, ct) for ct in range(4)]
                + [mk(emit_v, st) for st in range(4, 8)],
                1: [mk(emit_qk, 2, ct) for ct in range(4)]
                + [mk(emit_v, st) for st in range(8, 12)]
                + [mk(emit_oproj, 0, sti, oc) for sti in range(4) for oc in range(2)],
                2: [mk(emit_qk, 3, ct) for ct in range(2)]
                + [mk(emit_oproj, 1, sti, oc) for sti in range(4) for oc in range(2)],
                # strip 3 would otherwise run nearly filler-dry for 36 slots,
                # exposing the exp latency at every group boundary (and each
                # tiny gap drops the PE p-state): its own k tiles, the last v
                # strip and strip-2's output projection keep it fed
                3: [mk(emit_qk, 3, ct) for ct in range(2, 4)]
                + [mk(emit_v, st) for st in range(12, 16)]
                + [mk(emit_oproj, 2, sti, oc) for sti in range(4) for oc in range(2)],
            }

            for qs in range(QS):
                partials = []   # (ki, pref, regw, mid)
                valids = []
                for ki in range(KT):
                    st = spec[qs][ki]
                    if st[0] == "f":
                        valids.append(ki)
                    elif st[0] == "p":
                        partials.append((ki, st[1], st[2], st[3]))
                partials.sort(key=lambda t: t[1])
                # valid tiles first: the opening PV write must span the full
                # strip (pref 0), and partial tiles' mask multiplies then sit
                # well clear of the PV matmuls that consume them
                tiles = [(ki, 0, 0, -1) for ki in valids] + partials
                assert tiles and tiles[0][1] == 0, "need a full-width opener"
                sgs = [tiles[i : i + 2] for i in range(0, len(tiles), 2)]
                n_tiles = len(tiles)
                fq = fills[qs]
                n_slots = HPC * (len(sgs) + 1)
                reserve = 4 if qs == QS - 1 else 0
                # strip 0's fillers read x tokens 512-1023, which are still
                # in flight on the DMA stream when the strip opens
                defer = 4 if qs == 0 else 0
                # spread the filler pops evenly across the strip's slots: a
                # slot left dry exposes the exp latency AND drops the PE
                # p-state to half clock for the next ~3us.  Strip 3's first 6
                # units write the k tiles / v strip its own supergroups 6-7
                # read - they MUST be emitted before head 0 reaches slot 6,
                # so they front-load into slots 0-5.
                front = 6 if qs == QS - 1 else 0
                navail = max(len(fq) - reserve - front, 0)
                span = n_slots - defer - front
                sched = [0] * n_slots
                for i in range(front):
                    sched[i] += 1
                for i in range(navail):
                    sched[defer + front + i * span // max(navail, 1)] += 1
                slot = 0

                for h in range(HPC):
                    qT = qk[h // 2][64 * (h % 2) : 64 * (h % 2) + 64, :]
                    kT = qk[2 + h // 2][64 * (h % 2) : 64 * (h % 2) + 64, :]
                    po = ps_o.tile([65, 512], F32, tag="po")

                    pending = None
                    npv = 0
                    for g in range(len(sgs) + 1):
                        jobs = None
                        if g < len(sgs):
                            sg = sgs[g]
                            pst = ps_st.tile([128, 1024], F32, tag="pst")
                            pt = ptp.tile([128, 1024], F16, tag="pt")
                            jobs = []
                            off = 0
                            for ki, pref, regw, mid in sg:
                                w = 512 - pref
                                if off // 512 != (off + w - 1) // 512:
                                    # a single matmul's PSUM write must stay
                                    # inside one 2KB bank
                                    off = (off + 511) // 512 * 512
                                nc.tensor.matmul(
                                    pst[:, off : off + w],
                                    kT[:, 128 * ki : 128 * ki + 128],
                                    qT[:, 512 * qs + pref : 512 * qs + 512],
                                    start=True,
                                    stop=True,
                                )
                                jobs.append((pt, off, w, ki, pref, regw, mid))
                                off += w
                            # tight packing keeps this a single exp even for
                            # the partial (diagonal) pairs
                            nc.scalar.activation(
                                pt[:, 0:off],
                                pst[:, 0:off],
                                mybir.ActivationFunctionType.Exp,
                            )
                            # mask multiplies must stay on DVE: on Pool they
                            # force a microcode library swap (+quiescence
                            # sync, ~3us) around every partition_broadcast
                            # in the normalize chain
                            for pt_, o, w, ki, pref, regw, mid in jobs:
                                if mid >= 0:
                                    nc.vector.tensor_mul(
                                        pt_[:, o : o + regw],
                                        pt_[:, o : o + regw],
                                        mtiles[mid][:],
                                    )
                        for _ in range(min(sched[slot], max(len(fq) - reserve, 0))):
                            fq.pop(0)()
                        slot += 1
                        if pending is not None:
                            for pt_, o, w, ki, pref, regw, mid in pending:
                                vcol = 260 * (ki % 4) + 65 * h
                                nc.tensor.matmul(
                                    po[:, pref : pref + w],
                                    v_ext[ki // 4][:, vcol : vcol + 65],
                                    pt_[:, o : o + w],
                                    start=(npv == 0),
                                    stop=(npv == n_tiles - 1),
                                )
                                npv += 1
                        pending = jobs

                    if h == HPC - 1:
                        # flush leftover filler BEFORE the normalize chain
                        flush_engs = ["scalar", "vector"]
                        fi = 0
                        while fq:
                            fq.pop(0)(eng=flush_engs[fi % 2])
                            fi += 1
                        if qs == QS - 1:
                            # open the first two tail accumulations: their
                            # t=0 operand (heads 0-1) is long ready, and the
                            # 4 matmuls keep the PE clocked through the
                            # ~3.5us final normalize chain
                            for sti in range(2):
                                pop = ps_st.tile([128, 1024], F32, tag="pst")
                                for oc in range(2):
                                    nc.tensor.matmul(
                                        pop[:, 512 * oc : 512 * oc + 512],
                                        ot[0][3][:, 128 * sti : 128 * sti + 128],
                                        wo_t[:, 0, 512 * oc : 512 * oc + 512],
                                        start=True,
                                        stop=False,
                                    )
                                tail_pops.append(pop)
                    # normalize: row 64 of po is the softmax denominator
                    # (copy to SBUF first: the custom-DVE reciprocal misreads
                    # PSUM operands on hardware)
                    rden = nrm.tile([1, 512], F32, tag="rden")
                    nc.vector.tensor_copy(rden[:], po[64:65, :])
                    rrec = nrm.tile([1, 512], F32, tag="rrec")
                    nc.vector.reciprocal_approx_fast(rrec[:], rden[:])
                    rb = nrm.tile([64, 512], F32, tag="rb")
                    nc.gpsimd.partition_broadcast(rb[:], rrec[:])
                    nc.vector.tensor_mul(
                        ot[h // 2][qs][64 * (h % 2) : 64 * (h % 2) + 64, :],
                        po[0:64, :],
                        rb[:],
                    )

                while fq:
                    fq.pop(0)()

            # strip-3 output projection: each 512-token block's two column
            # halves accumulate in one rotating [128,1024] score buffer
            # (blocks 0-1 were opened before the final normalize), casts
            # alternate Act/DVE, and every block DMAs out on completion
            ob = osb[1]
            for sti in range(4):
                if sti < 2:
                    pop = tail_pops[sti]
                    for oc in range(2):
                        nc.tensor.matmul(
                            pop[:, 512 * oc : 512 * oc + 512],
                            ot[1][3][:, 128 * sti : 128 * sti + 128],
                            wo_t[:, 1, 512 * oc : 512 * oc + 512],
                            start=False,
                            stop=True,
                        )
                else:
                    pop = ps_st.tile([128, 1024], F32, tag="pst")
                    for oc in range(2):
                        for t in range(2):
                            nc.tensor.matmul(
                                pop[:, 512 * oc : 512 * oc + 512],
                                ot[t][3][:, 128 * sti : 128 * sti + 128],
                                wo_t[:, t, 512 * oc : 512 * oc + 512],
                                start=(t == 0),
                                stop=(t == 1),
                            )
                nc.scalar.copy(ob[:, sti, 0:512], pop[:, 0:512])
                nc.vector.tensor_copy(ob[:, sti, 512:1024], pop[:, 512:1024])
                nc.sync.dma_start(
                    out[:, 12 + sti : 13 + sti, :], ob[:, sti : sti + 1, :]
                )

    nc.finalize()
    return nc


_cache = {}


def _get_nc(key):
    if key not in _cache:
        spec, uregw = key
        _cache[key] = _build([list(r) for r in spec], list(uregw))
    return _cache[key]


def _tile_km(a):
    """[K*128, w] -> [128, K, w] partition-major contiguous."""
    k1, w = a.shape
    return np.ascontiguousarray(
        a.reshape(k1 // 128, 128, w).transpose(1, 0, 2)
    )


def _prepare(x, mask, w_qkv, w_out):
    """Host-side sharding. Returns (cache_key, in_maps)."""
    scale = 1.0 / np.sqrt(DH)

    keeps = [(mask[b] != 0).T.astype(np.float32) for b in range(B)]  # [k, q]
    keep_any = np.maximum.reduce(keeps)
    keep_all = np.minimum.reduce(keeps)

    uniq = {}
    uregw = []
    umask = []
    spec = []
    for qs in range(QS):
        row = []
        for ki in range(KT):
            blk_any = keep_any[128 * ki : 128 * ki + 128, 512 * qs : 512 * qs + 512]
            blk_all = keep_all[128 * ki : 128 * ki + 128, 512 * qs : 512 * qs + 512]
            if blk_any.max() == 0.0:
                row.append(("s",))
                continue
            if blk_all.min() == 1.0:
                row.append(("f",))
                continue
            colm = blk_any.max(axis=0)
            colv = blk_all.min(axis=0)
            nz = np.nonzero(colm)[0]
            pref = int(nz[0]) if len(nz) else 512
            mixed = np.nonzero(colv == 0)[0]
            end = int(mixed[-1]) + 1 if len(mixed) else pref
            regw = max(end - pref, 1)
            regs = tuple(
                k[128 * ki : 128 * ki + 128, 512 * qs + pref : 512 * qs + pref + regw]
                .astype(np.float16)
                .tobytes()
                for k in keeps
            )
            if regs not in uniq:
                uniq[regs] = len(uregw)
                uregw.append(regw)
                umask.append(
                    [
                        np.frombuffer(r, np.float16).reshape(128, regw)
                        for r in regs
                    ]
                )
            row.append(("p", pref, regw, uniq[regs]))
        spec.append(tuple(row))
    key = (tuple(spec), tuple(uregw))

    in_maps = []
    for c in range(NCORES):
        b, g = c // 4, c % 4
        heads = range(4 * g, 4 * g + 4)
        xT = _tile_km(_to_f16(x[b].T))            # [128, 8, 2048]
        wq = np.concatenate(
            [w_qkv[:, 64 * h : 64 * h + 64] for h in heads], axis=1
        ) * scale
        wk = np.concatenate(
            [w_qkv[:, D + 64 * h : D + 64 * h + 64] for h in heads], axis=1
        )
        wvv = np.concatenate(
            [w_qkv[:, 2 * D + 64 * h : 2 * D + 64 * h + 64] for h in heads], axis=1
        )
        woo = np.concatenate(
            [w_out[64 * h : 64 * h + 64, :] for h in heads], axis=0
        )
        wqk = _tile_km(_to_f16(np.concatenate([wq, wk], axis=1)))  # [128, 8, 512]
        if umask:
            mk = np.concatenate([r[b] for r in umask], axis=1).astype(np.float16)
        else:
            mk = np.zeros((128, 1), np.float16)
        im = {
            "wv": _tile_km(_to_f16(wvv)),
            "wo": _tile_km(_to_f16(np.ascontiguousarray(woo))),
            "maskp": np.ascontiguousarray(mk),
        }
        for i in range(4):
            im[f"wqc{i}"] = np.ascontiguousarray(wqk[:, 2 * i : 2 * i + 2, :])
            im[f"xtc{i}"] = np.ascontiguousarray(xT[:, 2 * i : 2 * i + 2, 0:512])
        for i in range(1, 4):
            im[f"xr{i}"] = np.ascontiguousarray(xT[:, :, 512 * i : 512 * i + 512])
        in_maps.append(im)
    return key, in_maps


def _unshuffle_out(o):
    """[128, 16, D] tile-major kernel output -> [S, D]."""
    return np.ascontiguousarray(o.transpose(1, 0, 2)).reshape(S, D)


def _run(x, mask, w_qkv, w_out, trace=False, trace_cores=None):
    key, in_maps = _prepare(x, mask, w_qkv, w_out)
    nc = _get_nc(key)
    res = run_bass_kernel_spmd(
        nc,
        in_maps,
        core_ids=list(range(NCORES)),
        trace=trace,
        trace_cores=trace_cores,
    )
    outs = np.stack(
        [
            sum(
                _unshuffle_out(res.results[4 * b + g]["out"].astype(np.float32))
                for g in range(4)
            )
            for b in range(B)
        ]
    )
    return outs.astype(np.float32), res


def kernel(x, mask, w_qkv, w_out):
    x = np.asarray(x, np.float32)
    mask = np.asarray(mask)
    w_qkv = np.asarray(w_qkv, np.float32)
    w_out = np.asarray(w_out, np.float32)
    out, _ = _run(x, mask, w_qkv, w_out)
    return out
